# revision 36
# baseline (speedup 1.0000x reference)
"""Multi-head attention (B=2, N=2048, C=768, H=12, DH=64) on 8 Trainium2 cores.

Sharding: data-parallel on batch (cores 0-3 -> b=0, cores 4-7 -> b=1),
tensor-parallel on heads within each group (3 heads/core: Wq/Wk/Wv column
slices, Wp row slices).  Each core emits its partial projection output
[N, C]; the host sums the 4 partials per batch and adds bp.

Per-core dataflow (feature-major, transpose-free, fp16 operands / fp32 psum):
  - host supplies xT = x[b].T  [C, N] in fp16; weight slices arrive
    pre-chunked [128, KC*W] so each loads with a single DMA; xT streams as
    12 kc-ordered half-chunk DMAs (the q/k sweeps chase the load)
  - qT,kT [64, N] per head = W.T @ xT, q/k sweeps interleaved per kc
    chunk (heads 0,1 full-M groups; the two 64-row leftovers of q and k
    merge into one M=128 group); when biases are nonzero they fold into
    K=1 ones-row matmuls (skipped entirely for the all-zero case);
    psum->SBUF casts run on the (else idle) scalar engine
  - v [N, 195] token-major with the softmax-denominator ones column baked
    into a zero-gap Wv layout ([v0|1|v1|1|v2|1]); its matmuls ride inside
    block 0's h01 pass (psum borrowed from the then-idle po slot) so the
    serial v phase disappears
  - ST [kj, qi] = kT.T-slice @ qT (scores, transposed); two K=64 matmuls
    packed on disjoint PE row halves stream CONCURRENTLY per [128,1024]
    psum tile (heads 0+1 paired; head 2 pairs even/odd kj)
  - ET = exp(ST - 4) one ACT op per [128,1024]; the ACT engine paces the
    attention inner loop (~1.19us/iter), all other work hides in its
    shadow
  - yT_aug[65, qi] = [v_h | 1].T @ ET accumulated over kj; row 64 = denom
  - normalize: reciprocal_approx_fast of the denom row (staged to SBUF
    first - its BITWISE_NOT seed misreads PSUM on HW), stride-0 DMA
    broadcast in-block, PE ones-broadcast at the tail (off the DMA
    latency path), fused mul-copy
  - out[qi, C] partial = yT @ Wp rows, split 512+256 wide and drained as
    HALF-units (one matmul each) into the next block's ACT-paced stream
    so each piece fits an iteration's PE slack; block 3 projects at the
    tail through two double-bank psum tiles with shared weight loads
"""

import math

import numpy as np

import concourse.bacc as bacc
import concourse.bass as bass
import concourse.mybir as mybir
import concourse.tile as tile
from concourse import bass_utils

B, N, C, H, DH = 2, 2048, 768, 12, 64
NCORES = 8
CPG = 4                  # cores per batch group
HPC = H // CPG           # heads per core = 3
MYC = HPC * DH           # per-core feature width = 192
VW = HPC * 65            # v row width with ones columns = 195
KC = C // 128            # contraction chunks = 6
NTT = N // 128           # token tiles = 16
QB = 512                 # qi block (psum bank width, fp32)
F32 = mybir.dt.float32
MMDT = mybir.dt.float16  # matmul operand dtype: 1cyc/row, 10-bit mantissa
AF = mybir.ActivationFunctionType
OP = mybir.AluOpType

EXP_SHIFT = -4.0         # exp(s + EXP_SHIFT); cancels between num and denom


def _bcast_parts(ap, nparts):
    """Partition-stride-0 broadcast view of a [1, F] AP (DMA source only)."""
    return bass.AP(tensor=ap.tensor, offset=ap.offset,
                   ap=[[0, nparts]] + [list(d) for d in ap.ap[1:]])


def _emit(nc, tc, pools, aps, with_bias=True):
    xT, wqA, wkA, wqkB, wv, wp = (
        aps["xT"], aps["wqA"], aps["wkA"], aps["wqkB"], aps["wv"], aps["wp"])
    bqA, bkA, bqkB, bvr, out = (
        aps["bqA"], aps["bkA"], aps["bqkB"], aps["bvr"], aps["out"])
    persist = pools["persist"]
    et_pool = pools["et"]
    small = pools["small"]
    ostage = pools["ostage"]
    dram_bc = pools["dram_bc"]

    # ---- persistent SBUF tensors ----
    xT_sb = persist.tile([128, KC * N], MMDT, tag="xT_sb")
    wqA_sb = persist.tile([128, KC * 128], MMDT, tag="wqA_sb")
    wkA_sb = persist.tile([128, KC * 128], MMDT, tag="wkA_sb")
    wqkB_sb = persist.tile([128, KC * 128], MMDT, tag="wqkB_sb")
    wv_sb = persist.tile([128, KC * VW], MMDT, tag="wv_sb")
    wpA = persist.tile([128, C], MMDT, tag="wpA")
    wpB = persist.tile([64, C], MMDT, tag="wpB")
    bq_row = persist.tile([1, 128], MMDT, tag="bq_row")
    bk_row = persist.tile([1, 128], MMDT, tag="bk_row")
    bqk_row = persist.tile([1, 128], MMDT, tag="bqk_row")
    bvr_sb = persist.tile([1, VW], MMDT, tag="bvr_sb")
    ones = persist.tile([1, 128], MMDT, tag="ones")
    ones512 = persist.tile([1, QB], MMDT, tag="ones512")
    shift_col = persist.tile([128, 1], F32, tag="shift_col")
    qTA = persist.tile([128, N], MMDT, tag="qTA")
    kTA = persist.tile([128, N], MMDT, tag="kTA")
    # head 2 k/q live duplicated on both partition halves (kj even/odd packing)
    qTB = persist.tile([128, N], MMDT, tag="qTB")
    kTB = persist.tile([128, N], MMDT, tag="kTB")
    v_sb = persist.tile([128, NTT * VW], MMDT, tag="v_sb")
    yTA = persist.tile([128, N], MMDT, tag="yTA")
    yTB = persist.tile([64, N], MMDT, tag="yTB")

    # ---- constants (vector) ----
    ones_f32 = persist.tile([1, QB], F32, tag="ones_f32")
    nc.vector.memset(ones_f32, 1.0)
    nc.vector.tensor_copy(out=ones, in_=ones_f32[:, 0:128])
    nc.vector.tensor_copy(out=ones512, in_=ones_f32)
    nc.vector.memset(shift_col, EXP_SHIFT)

    # ---- input DMAs; xT half-chunks round-robin across three engine
    # queues (each engine feeds its own DMA ring, tripling transfer
    # parallelism), kc-ordered; weights on the scalar queue ----
    nc.scalar.dma_start(out=wqA_sb, in_=wqA)
    nc.scalar.dma_start(out=wkA_sb, in_=wkA)
    nc.scalar.dma_start(out=wqkB_sb, in_=wqkB)
    # scalar queue stays clean after this (psum->SBUF copies run there);
    # xT on sync; the not-immediately-needed inputs on gpsimd
    for i in range(2 * KC):
        kc, h = i // 2, i % 2
        nc.sync.dma_start(
            out=xT_sb[:, kc * N + h * (N // 2):kc * N + (h + 1) * (N // 2)],
            in_=xT[kc * 128:(kc + 1) * 128,
                   h * (N // 2):(h + 1) * (N // 2)])
    nc.gpsimd.dma_start(out=wv_sb, in_=wv)
    nc.gpsimd.dma_start(out=wpA, in_=wp[0:128, :])
    nc.gpsimd.dma_start(out=wpB, in_=wp[128:MYC, :])
    nc.gpsimd.dma_start(out=bq_row, in_=bqA)
    nc.gpsimd.dma_start(out=bk_row, in_=bkA)
    nc.gpsimd.dma_start(out=bqk_row, in_=bqkB)
    nc.gpsimd.dma_start(out=bvr_sb, in_=bvr)

    # ---- phases 1+2: q/k/v projections (own PSUM pool, released after) ----
    with tc.tile_pool(name="ps_proj", bufs=2, space="PSUM") as ps_proj:
        # q and k sweeps interleaved per kc chunk so both finish right
        # behind the xT load; bias + psum->SBUF cast in nt order so st(0)'s
        # inputs (nt=0 slices of qTA/kTA) are ready first
        psq = [ps_proj.tile([128, QB], F32, tag="ps_qk", bufs=8,
                            name=f"ps_q{_i}") for _i in range(N // QB)]
        psk = [ps_proj.tile([128, QB], F32, tag="ps_qk", bufs=8,
                            name=f"ps_k{_i}") for _i in range(N // QB)]
        # PE warmup while the first DMAs land: ramps the pstate up (q's
        # kc=0 start=True matmul later resets this psum)
        for _ in range(5):
            nc.tensor.matmul(psq[0], ones[0:1, :], ones512, start=True,
                             stop=True)
        for kc in range(KC):  # kc outer: overlap the xT load
            for pss, wsb in ((psq, wqA_sb), (psk, wkA_sb)):
                for nt in range(N // QB):
                    nc.tensor.matmul(
                        pss[nt],
                        wsb[:, kc * 128:(kc + 1) * 128],
                        xT_sb[:, kc * N + nt * QB: kc * N + nt * QB + QB],
                        start=(kc == 0),
                        stop=(not with_bias and kc == KC - 1),
                    )
        for nt in range(N // QB):
            if with_bias:  # K=1 ones-row matmul adds the bias
                nc.tensor.matmul(psq[nt], bq_row, ones512,
                                 start=False, stop=True)
                nc.tensor.matmul(psk[nt], bk_row, ones512,
                                 start=False, stop=True)
            # q casts on scalar, k casts on vector: the two queues drain
            # in parallel so the qkB sweep gets its psum slots sooner
            nc.scalar.activation(out=qTA[:, nt * QB:(nt + 1) * QB],
                                 in_=psq[nt], func=AF.Copy, bias=0.0)
            nc.vector.tensor_copy(out=kTA[:, nt * QB:(nt + 1) * QB],
                                  in_=psk[nt])

        # merged leftover: psum rows 0:64 = q feats 128:192,
        # rows 64:128 = k feats 128:192
        pss = [ps_proj.tile([128, QB], F32, tag="ps_qk", bufs=8,
                            name=f"ps_qk{_i}")
               for _i in range(N // QB)]
        for kc in range(KC):
            for nt in range(N // QB):
                nc.tensor.matmul(
                    pss[nt],
                    wqkB_sb[:, kc * 128:(kc + 1) * 128],
                    xT_sb[:, kc * N + nt * QB: kc * N + nt * QB + QB],
                    start=(kc == 0),
                    stop=(not with_bias and kc == KC - 1),
                )
        for nt in range(N // QB):
            if with_bias:
                nc.tensor.matmul(pss[nt], bqk_row, ones512,
                                 start=False, stop=True)
            nc.scalar.activation(
                out=qTB[0:64, nt * QB:(nt + 1) * QB],
                in_=pss[nt][0:64, :], func=AF.Copy, bias=0.0)
            nc.vector.tensor_copy(
                out=kTB[0:64, nt * QB:(nt + 1) * QB],
                in_=pss[nt][64:128, :])
        # duplicate head-2 k/q onto partitions 64..127 (cross-partition: DMA)
        nc.sync.dma_start(out=qTB[64:128, :], in_=qTB[0:64, :])
        nc.sync.dma_start(out=kTB[64:128, :], in_=kTB[0:64, :])

    # ---- phase 3: attention; unit = (head-pair, qi block of 512) ----
    def vh_ap(kj, h):
        base = (kj * HPC + h) * 65
        return v_sb[:, base:base + 65]

    # ps_po declared FIRST so it lands on the projection phase's
    # earliest-freed psum bank: block 0's v matmuls (po slot) start as soon
    # as the first qkB cast completes instead of waiting for the last one
    with tc.tile_pool(name="ps_po", bufs=1, space="PSUM") as ps_po, \
         tc.tile_pool(name="ps_st", bufs=2, space="PSUM") as ps_st, \
         tc.tile_pool(name="ps_yt", bufs=3, space="PSUM") as ps_yt:

        def normalize(yt, ydst, q0, bc_ps=None, den_eng=None):
            # approx_fast's BITWISE_NOT seed misreads PSUM inputs on HW:
            # stage the denominator row to SBUF first
            den = small.tile([1, QB], F32, tag="den")
            if den_eng == "scalar":  # idle at the tail; runs parallel to DVE
                nc.scalar.activation(out=den, in_=yt[64:65, :], func=AF.Copy,
                                     bias=0.0)
            else:
                nc.vector.tensor_copy(out=den, in_=yt[64:65, :])
            rec = small.tile([1, QB], F32, tag="rec")
            nc.vector.reciprocal_approx_fast(out=rec, in_=den)
            if bc_ps is None:  # DMA round-trip broadcast (hidden in-block)
                dr = dram_bc.tile([1, QB], F32)
                nc.sync.dma_start(out=dr, in_=rec)
                bc = small.tile([64, QB], F32, tag="bc_sb")
                nc.sync.dma_start(out=bc, in_=_bcast_parts(dr, 64))
            else:  # PE ones-broadcast into psum (low-latency tail path)
                rec16 = small.tile([1, QB], MMDT, tag="rec16")
                nc.vector.tensor_copy(out=rec16, in_=rec)
                bc_p = bc_ps[0:64, 0:QB]
                nc.tensor.matmul(bc_p, ones[0:1, 0:64], rec16,
                                 start=True, stop=True)
                # stt allows only one PSUM input; idle scalar engine casts
                bc = small.tile([64, QB], F32, tag="bc_sb")
                nc.scalar.activation(out=bc, in_=bc_p, func=AF.Copy, bias=0.0)
            nc.vector.scalar_tensor_tensor(
                out=ydst[:, q0:q0 + QB], in0=yt[0:64, :], scalar=1.0, in1=bc,
                op0=OP.mult, op1=OP.mult,
            )

        # v production unit: emitted inside block 0's h01 pass so the PE's
        # ACT-slack absorbs it; psum borrowed from the (then idle) po slot
        def emit_v(nt):
            ps = ps_po.tile([128, QB], F32, tag="po", name=f"psv{nt}")
            psv = ps[:, 0:VW]
            for kc in range(KC):
                nc.tensor.matmul(
                    psv,
                    xT_sb[:, kc * N + nt * 128: kc * N + nt * 128 + 128],
                    wv_sb[:, kc * VW:(kc + 1) * VW],
                    start=(kc == 0), stop=False,
                )
            nc.tensor.matmul(psv, ones[0:1, 0:128], bvr_sb,
                             start=False, stop=True)
            nc.vector.tensor_copy(out=v_sb[:, nt * VW:(nt + 1) * VW], in_=psv)

        # Projection work for block qq arrives as HALF-units (one matmul
        # each) so a single iteration's ACT slack absorbs each piece;
        # block qq+1's emission drains them into the ACT-paced stream.
        proj_units = []
        po_map = {}

        NBW = ((0, QB), (QB, C))  # out-col splits: 512 + 256 wide

        def drain_proj(k=1, tile=None):
            for _ in range(min(k, len(proj_units))):
                kind, qt, nb, ob = proj_units.pop(0)
                c0, c1 = NBW[nb]
                if kind == "A":
                    po_t = tile
                    if po_t is None:
                        po_t = ps_po.tile([128, QB], F32, tag="po",
                                          name=f"po{qt}_{nb}")
                    po_map[(qt, nb)] = po_t
                    nc.tensor.matmul(po_t[:, 0:c1 - c0],
                                     yTA[:, qt * 128:(qt + 1) * 128],
                                     wpA[:, c0:c1], start=True, stop=False)
                else:
                    po_t = po_map.pop((qt, nb))
                    nc.tensor.matmul(po_t[:, 0:c1 - c0],
                                     yTB[0:64, qt * 128:(qt + 1) * 128],
                                     wpB[0:64, c0:c1], start=False, stop=True)
                    nc.vector.tensor_copy(out=ob[:, c0:c1],
                                          in_=po_t[:, 0:c1 - c0])
                    if nb == 1:
                        nc.sync.dma_start(out=out[qt * 128:(qt + 1) * 128, :],
                                          in_=ob)

        def queue_proj(qq):
            for qt in range(qq * 4, qq * 4 + 4):
                ob = ostage.tile([128, C], MMDT, tag="ob", name=f"ob{qt}")
                for nb in range(2):
                    proj_units.append(("A", qt, nb, ob))
                    proj_units.append(("B", qt, nb, ob))

        def h2_pass(qq):
            q0 = qq * QB
            yt2 = ps_yt.tile([65, QB], F32, tag="yt")
            prev = None
            for kp in range(NTT // 2):
                kj0, kj1 = 2 * kp, 2 * kp + 1
                st = ps_st.tile([128, 1024], F32, tag="st")
                nc.tensor.matmul(st[:, 0:QB],
                                 kTB[0:64, kj0 * 128:(kj0 + 1) * 128],
                                 qTB[0:64, q0:q0 + QB], start=True, stop=True)
                nc.tensor.matmul(st[:, QB:1024],
                                 kTB[64:128, kj1 * 128:(kj1 + 1) * 128],
                                 qTB[64:128, q0:q0 + QB], start=True, stop=True)
                et = et_pool.tile([128, 1024], MMDT)
                nc.scalar.activation(et, st, AF.Exp, bias=shift_col[:, :])
                if prev is not None:
                    pet, pkp = prev
                    nc.tensor.matmul(yt2, vh_ap(2 * pkp, 2), pet[:, 0:QB],
                                     start=(pkp == 0), stop=False)
                    nc.tensor.matmul(yt2, vh_ap(2 * pkp + 1, 2),
                                     pet[:, QB:1024], start=False, stop=False)
                prev = (et, kp)
                if kp >= 3:  # yT of qq-1 is normalized ~3 iters in
                    drain_proj(1)
            pet, pkp = prev
            nc.tensor.matmul(yt2, vh_ap(2 * pkp, 2), pet[:, 0:QB],
                             start=(pkp == 0), stop=False)
            nc.tensor.matmul(yt2, vh_ap(2 * pkp + 1, 2), pet[:, QB:1024],
                             start=False, stop=True)
            normalize(yt2, yTB[0:64, :], q0)

        def h01_pass(qq, with_v=False, tail=False):
            q0 = qq * QB
            yt0 = ps_yt.tile([65, QB], F32, tag="yt")
            yt1 = ps_yt.tile([65, QB], F32, tag="yt")
            prev = None
            for kj in range(NTT):
                if with_v:
                    emit_v(kj)
                st = ps_st.tile([128, 1024], F32, tag="st")
                nc.tensor.matmul(st[:, 0:QB],
                                 kTA[0:64, kj * 128:(kj + 1) * 128],
                                 qTA[0:64, q0:q0 + QB], start=True, stop=True)
                nc.tensor.matmul(st[:, QB:1024],
                                 kTA[64:128, kj * 128:(kj + 1) * 128],
                                 qTA[64:128, q0:q0 + QB], start=True, stop=True)
                et = et_pool.tile([128, 1024], MMDT)
                nc.scalar.activation(et, st, AF.Exp, bias=shift_col[:, :])
                if prev is not None:
                    pet, pkj = prev
                    nc.tensor.matmul(yt0, vh_ap(pkj, 0), pet[:, 0:QB],
                                     start=(pkj == 0), stop=False)
                    nc.tensor.matmul(yt1, vh_ap(pkj, 1), pet[:, QB:1024],
                                     start=(pkj == 0), stop=False)
                prev = (et, kj)
                drain_proj(1)
            pet, pkj = prev
            nc.tensor.matmul(yt0, vh_ap(pkj, 0), pet[:, 0:QB],
                             start=False, stop=True)
            nc.tensor.matmul(yt1, vh_ap(pkj, 1), pet[:, QB:1024],
                             start=False, stop=True)
            if not tail:
                normalize(yt0, yTA[0:64, :], q0)
                normalize(yt1, yTA[64:128, :], q0)
            else:  # tail: PE broadcast skips the DMA round-trip latency
                bc0 = ps_yt.tile([65, QB], F32, tag="yt", name="bc0")
                normalize(yt0, yTA[0:64, :], q0, bc_ps=bc0, den_eng="scalar")
                bc1 = ps_po.tile([128, QB], F32, tag="po", name="bc1")
                normalize(yt1, yTA[64:128, :], q0, bc_ps=bc1)

        # block 0: h01 first, with v production riding in its ACT slack
        h01_pass(0, with_v=True)
        h2_pass(0)
        queue_proj(0)
        for qq in range(1, 4):
            h2_pass(qq)
            h01_pass(qq, tail=(qq == 3))
            if qq < 3:
                queue_proj(qq)

        # block 3's projection (the tail): per qt one double-bank psum tile
        # holds both outputs, the two yTA (then yTB) matmuls share a weight
        # load, and the halves DMA out as each cast completes
        tis = [ps_st.tile([128, 1024], F32, tag="st", name="tp0"),
               ps_st.tile([128, 1024], F32, tag="st", name="tp1")]
        for qt in range(12, 16):
            t = tis[qt % 2]
            pos = (t[:, 0:QB], t[:, QB:QB + 256])
            for nb in range(2):
                c0, c1 = NBW[nb]
                nc.tensor.matmul(pos[nb], yTA[:, qt * 128:(qt + 1) * 128],
                                 wpA[:, c0:c1], start=True, stop=False)
            for nb in range(2):
                c0, c1 = NBW[nb]
                nc.tensor.matmul(pos[nb], yTB[0:64, qt * 128:(qt + 1) * 128],
                                 wpB[0:64, c0:c1], start=False, stop=True)
            ob = ostage.tile([128, C], MMDT, tag="ob", name=f"obt{qt}")
            nc.vector.tensor_copy(out=ob[:, 0:QB], in_=pos[0])
            nc.sync.dma_start(out=out[qt * 128:(qt + 1) * 128, 0:QB],
                              in_=ob[:, 0:QB])
            nc.scalar.activation(out=ob[:, QB:C], in_=pos[1], func=AF.Copy,
                                 bias=0.0)
            nc.sync.dma_start(out=out[qt * 128:(qt + 1) * 128, QB:C],
                              in_=ob[:, QB:C])


def _build_program(with_bias=True):
    nc = bacc.Bacc("TRN2", target_bir_lowering=False, debug=False,
                   num_devices=NCORES)
    aps = {
        "xT": nc.dram_tensor("xT", [C, N], MMDT, kind="ExternalInput").ap(),
        # weights arrive pre-chunked: [128, KC*W] with chunk kc at cols
        # kc*W:(kc+1)*W   (host does the (6,128,W)->(128,6,W) transpose)
        "wqA": nc.dram_tensor("wqA", [128, KC * 128], MMDT,
                              kind="ExternalInput").ap(),
        "wkA": nc.dram_tensor("wkA", [128, KC * 128], MMDT,
                              kind="ExternalInput").ap(),
        "wqkB": nc.dram_tensor("wqkB", [128, KC * 128], MMDT,
                               kind="ExternalInput").ap(),
        "wv": nc.dram_tensor("wv", [128, KC * VW], MMDT,
                             kind="ExternalInput").ap(),
        "wp": nc.dram_tensor("wp", [MYC, C], MMDT, kind="ExternalInput").ap(),
        "bqA": nc.dram_tensor("bqA", [1, 128], MMDT,
                              kind="ExternalInput").ap(),
        "bkA": nc.dram_tensor("bkA", [1, 128], MMDT,
                              kind="ExternalInput").ap(),
        "bqkB": nc.dram_tensor("bqkB", [1, 128], MMDT,
                               kind="ExternalInput").ap(),
        "bvr": nc.dram_tensor("bvr", [1, VW], MMDT, kind="ExternalInput").ap(),
        "out": nc.dram_tensor("out", [N, C], MMDT,
                              kind="ExternalOutput").ap(),
    }
    with tile.TileContext(nc) as tc:
        import contextlib
        with contextlib.ExitStack() as ctx:
            pools = {
                "persist": ctx.enter_context(tc.tile_pool(name="persist", bufs=1)),
                "et": ctx.enter_context(tc.tile_pool(name="et", bufs=4)),
                "small": ctx.enter_context(tc.tile_pool(name="small", bufs=2)),
                "ostage": ctx.enter_context(tc.tile_pool(name="ostage", bufs=3)),
                "dram_bc": ctx.enter_context(
                    tc.tile_pool(name="dram_bc", bufs=2, space="DRAM")),
            }
            _emit(nc, tc, pools, aps, with_bias=with_bias)
    nc.compile()
    return nc


_PROGRAM_CACHE = {}


def _get_program(with_bias=True):
    key = f"nc{int(with_bias)}"
    if key not in _PROGRAM_CACHE:
        _PROGRAM_CACHE[key] = _build_program(with_bias=with_bias)
    return _PROGRAM_CACHE[key]


def _chunked(w):
    """[C, W] -> [128, KC*W]: chunk kc lands at columns kc*W:(kc+1)*W."""
    wc = np.ascontiguousarray(w)
    return wc.reshape(KC, 128, w.shape[1]).transpose(1, 0, 2).reshape(
        128, KC * w.shape[1])


def make_in_maps(x, Wq, bq, Wk, bk, Wv, bv, Wp, bp):
    scale = 1.0 / math.sqrt(DH)
    xTb = [np.ascontiguousarray(x[b].T) for b in range(B)]
    wire = mybir.dt.np(MMDT)
    in_maps = []
    for c in range(NCORES):
        b, hg = c // CPG, c % CPG
        cols = slice(hg * MYC, (hg + 1) * MYC)
        wq_c = Wq[:, cols] * np.float32(scale)
        wk_c = Wk[:, cols]
        wv_c = Wv[:, cols]
        # zero-gap wv: [v0 | 1-col | v1 | 1-col | v2 | 1-col]; bias row gets
        # the ones so psum comes out in v_sb layout directly
        wv_aug = np.zeros((C, VW), np.float32)
        bv_aug = np.zeros((1, VW), np.float32)
        for h in range(HPC):
            wv_aug[:, h * 65:h * 65 + 64] = wv_c[:, h * DH:(h + 1) * DH]
            bv_aug[0, h * 65:h * 65 + 64] = bv[cols][h * DH:(h + 1) * DH]
            bv_aug[0, h * 65 + 64] = 1.0
        in_maps.append({
            "xT": xTb[b].astype(wire),
            "wqA": _chunked(wq_c[:, 0:128]).astype(wire),
            "wkA": _chunked(wk_c[:, 0:128]).astype(wire),
            "wqkB": _chunked(np.concatenate([wq_c[:, 128:], wk_c[:, 128:]],
                                            axis=1)).astype(wire),
            "wv": _chunked(wv_aug).astype(wire),
            "wp": np.ascontiguousarray(Wp[cols, :]).astype(wire),
            "bqA": (bq[cols][0:128] * np.float32(scale)).reshape(1, 128)
                   .astype(wire),
            "bkA": bk[cols][0:128].reshape(1, 128).astype(wire),
            "bqkB": np.concatenate([bq[cols][128:] * np.float32(scale),
                                    bk[cols][128:]]).reshape(1, 128)
                    .astype(wire),
            "bvr": bv_aug.astype(wire),
        })
    return in_maps


def assemble(results, bp):
    out = np.empty((B, N, C), np.float32)
    for b in range(B):
        acc = results[b * CPG]["out"].astype(np.float64)
        for c in range(b * CPG + 1, (b + 1) * CPG):
            acc = acc + results[c]["out"]
        out[b] = (acc + bp.astype(np.float64)).astype(np.float32)
    return out


def kernel(x, Wq, bq, Wk, bk, Wv, bv, Wp, bp, **extra_kwargs):
    x = np.asarray(x, np.float32)
    Wq = np.asarray(Wq, np.float32)
    Wk = np.asarray(Wk, np.float32)
    Wv = np.asarray(Wv, np.float32)
    Wp = np.asarray(Wp, np.float32)
    bq = np.asarray(bq, np.float32)
    bk = np.asarray(bk, np.float32)
    bv = np.asarray(bv, np.float32)
    bp = np.asarray(bp, np.float32)

    with_bias = bool(np.any(bq) or np.any(bk))
    nc = _get_program(with_bias=with_bias)
    in_maps = make_in_maps(x, Wq, bq, Wk, bk, Wv, bv, Wp, bp)
    res = bass_utils.run_bass_kernel_spmd(nc, in_maps,
                                          core_ids=list(range(NCORES)))
    return assemble(res.results, bp)


# revision 37
# speedup vs baseline: 1.0082x; 1.0082x over previous
"""Multi-head attention (B=2, N=2048, C=768, H=12, DH=64) on 8 Trainium2 cores.

Sharding: data-parallel on batch (cores 0-3 -> b=0, cores 4-7 -> b=1),
tensor-parallel on heads within each group (3 heads/core: Wq/Wk/Wv column
slices, Wp row slices).  Each core emits its partial projection output
[N, C]; the host sums the 4 partials per batch and adds bp.

Per-core dataflow (feature-major, transpose-free, fp16 operands / fp32 psum):
  - host supplies xT = x[b].T  [C, N] in fp16; weight slices arrive
    pre-chunked [128, KC*W] so each loads with a single DMA; xT streams as
    12 kc-ordered half-chunk DMAs (the q/k sweeps chase the load)
  - qT,kT [64, N] per head = W.T @ xT, q/k sweeps interleaved per kc
    chunk (heads 0,1 full-M groups; the two 64-row leftovers of q and k
    merge into one M=128 group); when biases are nonzero they fold into
    K=1 ones-row matmuls (skipped entirely for the all-zero case);
    psum->SBUF casts run on the (else idle) scalar engine
  - v [N, 195] token-major with the softmax-denominator ones column baked
    into a zero-gap Wv layout ([v0|1|v1|1|v2|1]); its matmuls ride inside
    block 0's h01 pass (psum borrowed from the then-idle po slot) so the
    serial v phase disappears
  - ST [kj, qi] = kT.T-slice @ qT (scores, transposed); two K=64 matmuls
    packed on disjoint PE row halves stream CONCURRENTLY per [128,1024]
    psum tile (heads 0+1 paired; head 2 pairs even/odd kj)
  - ET = exp(ST - 4) one ACT op per [128,1024]; the ACT engine paces the
    attention inner loop (~1.19us/iter), all other work hides in its
    shadow
  - yT_aug[65, qi] = [v_h | 1].T @ ET accumulated over kj; row 64 = denom
  - normalize: reciprocal_approx_fast of the denom row (staged to SBUF
    first - its BITWISE_NOT seed misreads PSUM on HW), stride-0 DMA
    broadcast in-block, PE ones-broadcast at the tail (off the DMA
    latency path), fused mul-copy
  - out[qi, C] partial = yT @ Wp rows, split 512+256 wide and drained as
    HALF-units (one matmul each) into the next block's ACT-paced stream
    so each piece fits an iteration's PE slack; block 3 projects at the
    tail through two double-bank psum tiles with shared weight loads
"""

import math

import numpy as np

import concourse.bacc as bacc
import concourse.bass as bass
import concourse.mybir as mybir
import concourse.tile as tile
from concourse import bass_utils

B, N, C, H, DH = 2, 2048, 768, 12, 64
NCORES = 8
CPG = 4                  # cores per batch group
HPC = H // CPG           # heads per core = 3
MYC = HPC * DH           # per-core feature width = 192
VW = HPC * 65            # v row width with ones columns = 195
KC = C // 128            # contraction chunks = 6
NTT = N // 128           # token tiles = 16
QB = 512                 # qi block (psum bank width, fp32)
F32 = mybir.dt.float32
MMDT = mybir.dt.float16  # matmul operand dtype: 1cyc/row, 10-bit mantissa
AF = mybir.ActivationFunctionType
OP = mybir.AluOpType

EXP_SHIFT = -4.0         # exp(s + EXP_SHIFT); cancels between num and denom


def _bcast_parts(ap, nparts):
    """Partition-stride-0 broadcast view of a [1, F] AP (DMA source only)."""
    return bass.AP(tensor=ap.tensor, offset=ap.offset,
                   ap=[[0, nparts]] + [list(d) for d in ap.ap[1:]])


def _emit(nc, tc, pools, aps, with_bias=True):
    xT, wqA, wkA, wqkB, wv, wp = (
        aps["xT"], aps["wqA"], aps["wkA"], aps["wqkB"], aps["wv"], aps["wp"])
    bqA, bkA, bqkB, bvr, out = (
        aps["bqA"], aps["bkA"], aps["bqkB"], aps["bvr"], aps["out"])
    persist = pools["persist"]
    et_pool = pools["et"]
    small = pools["small"]
    ostage = pools["ostage"]
    dram_bc = pools["dram_bc"]

    # ---- persistent SBUF tensors ----
    xT_sb = persist.tile([128, KC * N], MMDT, tag="xT_sb")
    wqA_sb = persist.tile([128, KC * 128], MMDT, tag="wqA_sb")
    wkA_sb = persist.tile([128, KC * 128], MMDT, tag="wkA_sb")
    wqkB_sb = persist.tile([128, KC * 128], MMDT, tag="wqkB_sb")
    wv_sb = persist.tile([128, KC * VW], MMDT, tag="wv_sb")
    wpA = persist.tile([128, C], MMDT, tag="wpA")
    wpB = persist.tile([64, C], MMDT, tag="wpB")
    bq_row = persist.tile([1, 128], MMDT, tag="bq_row")
    bk_row = persist.tile([1, 128], MMDT, tag="bk_row")
    bqk_row = persist.tile([1, 128], MMDT, tag="bqk_row")
    bvr_sb = persist.tile([1, VW], MMDT, tag="bvr_sb")
    ones = persist.tile([1, 128], MMDT, tag="ones")
    ones512 = persist.tile([1, QB], MMDT, tag="ones512")
    shift_col = persist.tile([128, 1], F32, tag="shift_col")
    qTA = persist.tile([128, N], MMDT, tag="qTA")
    kTA = persist.tile([128, N], MMDT, tag="kTA")
    # head 2 k/q live duplicated on both partition halves (kj even/odd packing)
    qTB = persist.tile([128, N], MMDT, tag="qTB")
    kTB = persist.tile([128, N], MMDT, tag="kTB")
    v_sb = persist.tile([128, NTT * VW], MMDT, tag="v_sb")
    yTA = persist.tile([128, N], MMDT, tag="yTA")
    yTB = persist.tile([64, N], MMDT, tag="yTB")

    # ---- constants (vector) ----
    ones_f32 = persist.tile([1, QB], F32, tag="ones_f32")
    nc.vector.memset(ones_f32, 1.0)
    nc.vector.tensor_copy(out=ones, in_=ones_f32[:, 0:128])
    nc.vector.tensor_copy(out=ones512, in_=ones_f32)
    nc.vector.memset(shift_col, EXP_SHIFT)

    # ---- input DMAs; xT half-chunks round-robin across three engine
    # queues (each engine feeds its own DMA ring, tripling transfer
    # parallelism), kc-ordered; weights on the scalar queue ----
    nc.scalar.dma_start(out=wqA_sb, in_=wqA)
    nc.scalar.dma_start(out=wkA_sb, in_=wkA)
    nc.scalar.dma_start(out=wqkB_sb, in_=wqkB)
    # scalar queue stays clean after this (psum->SBUF copies run there);
    # xT split 8 pieces on sync + 4 on gpsimd: each issuing queue feeds its
    # own ~8 DMA rings, so two queues engage more engines in parallel
    for i in range(2 * KC):
        kc, h = i // 2, i % 2
        eng = nc.gpsimd if i % 3 == 2 else nc.sync
        eng.dma_start(
            out=xT_sb[:, kc * N + h * (N // 2):kc * N + (h + 1) * (N // 2)],
            in_=xT[kc * 128:(kc + 1) * 128,
                   h * (N // 2):(h + 1) * (N // 2)])
    nc.gpsimd.dma_start(out=wv_sb, in_=wv)
    nc.gpsimd.dma_start(out=wpA, in_=wp[0:128, :])
    nc.gpsimd.dma_start(out=wpB, in_=wp[128:MYC, :])
    nc.gpsimd.dma_start(out=bq_row, in_=bqA)
    nc.gpsimd.dma_start(out=bk_row, in_=bkA)
    nc.gpsimd.dma_start(out=bqk_row, in_=bqkB)
    nc.gpsimd.dma_start(out=bvr_sb, in_=bvr)

    # ---- phases 1+2: q/k/v projections (own PSUM pool, released after) ----
    with tc.tile_pool(name="ps_proj", bufs=2, space="PSUM") as ps_proj:
        # q and k sweeps interleaved per kc chunk so both finish right
        # behind the xT load; bias + psum->SBUF cast in nt order so st(0)'s
        # inputs (nt=0 slices of qTA/kTA) are ready first
        psq = [ps_proj.tile([128, QB], F32, tag="ps_qk", bufs=8,
                            name=f"ps_q{_i}") for _i in range(N // QB)]
        psk = [ps_proj.tile([128, QB], F32, tag="ps_qk", bufs=8,
                            name=f"ps_k{_i}") for _i in range(N // QB)]
        # PE warmup while the first DMAs land: ramps the pstate up (q's
        # kc=0 start=True matmul later resets this psum)
        for _ in range(5):
            nc.tensor.matmul(psq[0], ones[0:1, :], ones512, start=True,
                             stop=True)
        for kc in range(KC):  # kc outer: overlap the xT load
            for pss, wsb in ((psq, wqA_sb), (psk, wkA_sb)):
                for nt in range(N // QB):
                    nc.tensor.matmul(
                        pss[nt],
                        wsb[:, kc * 128:(kc + 1) * 128],
                        xT_sb[:, kc * N + nt * QB: kc * N + nt * QB + QB],
                        start=(kc == 0),
                        stop=(not with_bias and kc == KC - 1),
                    )
        for nt in range(N // QB):
            if with_bias:  # K=1 ones-row matmul adds the bias
                nc.tensor.matmul(psq[nt], bq_row, ones512,
                                 start=False, stop=True)
                nc.tensor.matmul(psk[nt], bk_row, ones512,
                                 start=False, stop=True)
            # q casts on scalar, k casts on vector: the two queues drain
            # in parallel so the qkB sweep gets its psum slots sooner
            nc.scalar.activation(out=qTA[:, nt * QB:(nt + 1) * QB],
                                 in_=psq[nt], func=AF.Copy, bias=0.0)
            nc.vector.tensor_copy(out=kTA[:, nt * QB:(nt + 1) * QB],
                                  in_=psk[nt])

        # merged leftover: psum rows 0:64 = q feats 128:192,
        # rows 64:128 = k feats 128:192
        pss = [ps_proj.tile([128, QB], F32, tag="ps_qk", bufs=8,
                            name=f"ps_qk{_i}")
               for _i in range(N // QB)]
        for kc in range(KC):
            for nt in range(N // QB):
                nc.tensor.matmul(
                    pss[nt],
                    wqkB_sb[:, kc * 128:(kc + 1) * 128],
                    xT_sb[:, kc * N + nt * QB: kc * N + nt * QB + QB],
                    start=(kc == 0),
                    stop=(not with_bias and kc == KC - 1),
                )
        for nt in range(N // QB):
            if with_bias:
                nc.tensor.matmul(pss[nt], bqk_row, ones512,
                                 start=False, stop=True)
            nc.scalar.activation(
                out=qTB[0:64, nt * QB:(nt + 1) * QB],
                in_=pss[nt][0:64, :], func=AF.Copy, bias=0.0)
            nc.vector.tensor_copy(
                out=kTB[0:64, nt * QB:(nt + 1) * QB],
                in_=pss[nt][64:128, :])
        # duplicate head-2 k/q onto partitions 64..127 (cross-partition: DMA)
        nc.sync.dma_start(out=qTB[64:128, :], in_=qTB[0:64, :])
        nc.sync.dma_start(out=kTB[64:128, :], in_=kTB[0:64, :])

    # ---- phase 3: attention; unit = (head-pair, qi block of 512) ----
    def vh_ap(kj, h):
        base = (kj * HPC + h) * 65
        return v_sb[:, base:base + 65]

    # ps_po declared FIRST so it lands on the projection phase's
    # earliest-freed psum bank: block 0's v matmuls (po slot) start as soon
    # as the first qkB cast completes instead of waiting for the last one
    with tc.tile_pool(name="ps_po", bufs=1, space="PSUM") as ps_po, \
         tc.tile_pool(name="ps_st", bufs=2, space="PSUM") as ps_st, \
         tc.tile_pool(name="ps_yt", bufs=3, space="PSUM") as ps_yt:

        def normalize(yt, ydst, q0, bc_ps=None, den_eng=None):
            # approx_fast's BITWISE_NOT seed misreads PSUM inputs on HW:
            # stage the denominator row to SBUF first
            den = small.tile([1, QB], F32, tag="den")
            if den_eng == "scalar":  # idle at the tail; runs parallel to DVE
                nc.scalar.activation(out=den, in_=yt[64:65, :], func=AF.Copy,
                                     bias=0.0)
            else:
                nc.vector.tensor_copy(out=den, in_=yt[64:65, :])
            rec = small.tile([1, QB], F32, tag="rec")
            nc.vector.reciprocal_approx_fast(out=rec, in_=den)
            if bc_ps is None:  # DMA round-trip broadcast (hidden in-block)
                dr = dram_bc.tile([1, QB], F32)
                nc.sync.dma_start(out=dr, in_=rec)
                bc = small.tile([64, QB], F32, tag="bc_sb")
                nc.sync.dma_start(out=bc, in_=_bcast_parts(dr, 64))
            else:  # PE ones-broadcast into psum (low-latency tail path)
                rec16 = small.tile([1, QB], MMDT, tag="rec16")
                nc.vector.tensor_copy(out=rec16, in_=rec)
                bc_p = bc_ps[0:64, 0:QB]
                nc.tensor.matmul(bc_p, ones[0:1, 0:64], rec16,
                                 start=True, stop=True)
                # stt allows only one PSUM input; idle scalar engine casts
                bc = small.tile([64, QB], F32, tag="bc_sb")
                nc.scalar.activation(out=bc, in_=bc_p, func=AF.Copy, bias=0.0)
            nc.vector.scalar_tensor_tensor(
                out=ydst[:, q0:q0 + QB], in0=yt[0:64, :], scalar=1.0, in1=bc,
                op0=OP.mult, op1=OP.mult,
            )

        # v production unit: emitted inside block 0's h01 pass so the PE's
        # ACT-slack absorbs it; psum borrowed from the (then idle) po slot
        def emit_v(nt):
            ps = ps_po.tile([128, QB], F32, tag="po", name=f"psv{nt}")
            psv = ps[:, 0:VW]
            for kc in range(KC):
                nc.tensor.matmul(
                    psv,
                    xT_sb[:, kc * N + nt * 128: kc * N + nt * 128 + 128],
                    wv_sb[:, kc * VW:(kc + 1) * VW],
                    start=(kc == 0), stop=False,
                )
            nc.tensor.matmul(psv, ones[0:1, 0:128], bvr_sb,
                             start=False, stop=True)
            nc.vector.tensor_copy(out=v_sb[:, nt * VW:(nt + 1) * VW], in_=psv)

        # Projection work for block qq arrives as HALF-units (one matmul
        # each) so a single iteration's ACT slack absorbs each piece;
        # block qq+1's emission drains them into the ACT-paced stream.
        proj_units = []
        po_map = {}

        NBW = ((0, QB), (QB, C))  # out-col splits: 512 + 256 wide

        def drain_proj(k=1, tile=None):
            for _ in range(min(k, len(proj_units))):
                kind, qt, nb, ob = proj_units.pop(0)
                c0, c1 = NBW[nb]
                if kind == "A":
                    po_t = tile
                    if po_t is None:
                        po_t = ps_po.tile([128, QB], F32, tag="po",
                                          name=f"po{qt}_{nb}")
                    po_map[(qt, nb)] = po_t
                    nc.tensor.matmul(po_t[:, 0:c1 - c0],
                                     yTA[:, qt * 128:(qt + 1) * 128],
                                     wpA[:, c0:c1], start=True, stop=False)
                else:
                    po_t = po_map.pop((qt, nb))
                    nc.tensor.matmul(po_t[:, 0:c1 - c0],
                                     yTB[0:64, qt * 128:(qt + 1) * 128],
                                     wpB[0:64, c0:c1], start=False, stop=True)
                    nc.vector.tensor_copy(out=ob[:, c0:c1],
                                          in_=po_t[:, 0:c1 - c0])
                    if nb == 1:
                        nc.sync.dma_start(out=out[qt * 128:(qt + 1) * 128, :],
                                          in_=ob)

        def queue_proj(qq):
            for qt in range(qq * 4, qq * 4 + 4):
                ob = ostage.tile([128, C], MMDT, tag="ob", name=f"ob{qt}")
                for nb in range(2):
                    proj_units.append(("A", qt, nb, ob))
                    proj_units.append(("B", qt, nb, ob))

        def h2_pass(qq):
            q0 = qq * QB
            yt2 = ps_yt.tile([65, QB], F32, tag="yt")
            prev = None
            for kp in range(NTT // 2):
                kj0, kj1 = 2 * kp, 2 * kp + 1
                st = ps_st.tile([128, 1024], F32, tag="st")
                nc.tensor.matmul(st[:, 0:QB],
                                 kTB[0:64, kj0 * 128:(kj0 + 1) * 128],
                                 qTB[0:64, q0:q0 + QB], start=True, stop=True)
                nc.tensor.matmul(st[:, QB:1024],
                                 kTB[64:128, kj1 * 128:(kj1 + 1) * 128],
                                 qTB[64:128, q0:q0 + QB], start=True, stop=True)
                et = et_pool.tile([128, 1024], MMDT)
                nc.scalar.activation(et, st, AF.Exp, bias=shift_col[:, :])
                if prev is not None:
                    pet, pkp = prev
                    nc.tensor.matmul(yt2, vh_ap(2 * pkp, 2), pet[:, 0:QB],
                                     start=(pkp == 0), stop=False)
                    nc.tensor.matmul(yt2, vh_ap(2 * pkp + 1, 2),
                                     pet[:, QB:1024], start=False, stop=False)
                prev = (et, kp)
                if kp >= 3:  # yT of qq-1 is normalized ~3 iters in
                    drain_proj(1)
            pet, pkp = prev
            nc.tensor.matmul(yt2, vh_ap(2 * pkp, 2), pet[:, 0:QB],
                             start=(pkp == 0), stop=False)
            nc.tensor.matmul(yt2, vh_ap(2 * pkp + 1, 2), pet[:, QB:1024],
                             start=False, stop=True)
            normalize(yt2, yTB[0:64, :], q0)

        def h01_pass(qq, with_v=False, tail=False):
            q0 = qq * QB
            yt0 = ps_yt.tile([65, QB], F32, tag="yt")
            yt1 = ps_yt.tile([65, QB], F32, tag="yt")
            prev = None
            for kj in range(NTT):
                if with_v:
                    emit_v(kj)
                st = ps_st.tile([128, 1024], F32, tag="st")
                nc.tensor.matmul(st[:, 0:QB],
                                 kTA[0:64, kj * 128:(kj + 1) * 128],
                                 qTA[0:64, q0:q0 + QB], start=True, stop=True)
                nc.tensor.matmul(st[:, QB:1024],
                                 kTA[64:128, kj * 128:(kj + 1) * 128],
                                 qTA[64:128, q0:q0 + QB], start=True, stop=True)
                et = et_pool.tile([128, 1024], MMDT)
                nc.scalar.activation(et, st, AF.Exp, bias=shift_col[:, :])
                if prev is not None:
                    pet, pkj = prev
                    nc.tensor.matmul(yt0, vh_ap(pkj, 0), pet[:, 0:QB],
                                     start=(pkj == 0), stop=False)
                    nc.tensor.matmul(yt1, vh_ap(pkj, 1), pet[:, QB:1024],
                                     start=(pkj == 0), stop=False)
                prev = (et, kj)
                drain_proj(1)
            pet, pkj = prev
            nc.tensor.matmul(yt0, vh_ap(pkj, 0), pet[:, 0:QB],
                             start=False, stop=True)
            nc.tensor.matmul(yt1, vh_ap(pkj, 1), pet[:, QB:1024],
                             start=False, stop=True)
            if not tail:
                normalize(yt0, yTA[0:64, :], q0)
                normalize(yt1, yTA[64:128, :], q0)
            else:  # tail: PE broadcast skips the DMA round-trip latency
                bc0 = ps_yt.tile([65, QB], F32, tag="yt", name="bc0")
                normalize(yt0, yTA[0:64, :], q0, bc_ps=bc0, den_eng="scalar")
                bc1 = ps_po.tile([128, QB], F32, tag="po", name="bc1")
                normalize(yt1, yTA[64:128, :], q0, bc_ps=bc1)

        # block 0: h01 first, with v production riding in its ACT slack
        h01_pass(0, with_v=True)
        h2_pass(0)
        queue_proj(0)
        for qq in range(1, 4):
            h2_pass(qq)
            h01_pass(qq, tail=(qq == 3))
            if qq < 3:
                queue_proj(qq)

        # block 3's projection (the tail): per qt one double-bank psum tile
        # holds both outputs, the two yTA (then yTB) matmuls share a weight
        # load, and the halves DMA out as each cast completes
        tis = [ps_st.tile([128, 1024], F32, tag="st", name="tp0"),
               ps_st.tile([128, 1024], F32, tag="st", name="tp1")]
        for qt in range(12, 16):
            t = tis[qt % 2]
            pos = (t[:, 0:QB], t[:, QB:QB + 256])
            for nb in range(2):
                c0, c1 = NBW[nb]
                nc.tensor.matmul(pos[nb], yTA[:, qt * 128:(qt + 1) * 128],
                                 wpA[:, c0:c1], start=True, stop=False)
            for nb in range(2):
                c0, c1 = NBW[nb]
                nc.tensor.matmul(pos[nb], yTB[0:64, qt * 128:(qt + 1) * 128],
                                 wpB[0:64, c0:c1], start=False, stop=True)
            ob = ostage.tile([128, C], MMDT, tag="ob", name=f"obt{qt}")
            nc.vector.tensor_copy(out=ob[:, 0:QB], in_=pos[0])
            nc.sync.dma_start(out=out[qt * 128:(qt + 1) * 128, 0:QB],
                              in_=ob[:, 0:QB])
            nc.scalar.activation(out=ob[:, QB:C], in_=pos[1], func=AF.Copy,
                                 bias=0.0)
            nc.sync.dma_start(out=out[qt * 128:(qt + 1) * 128, QB:C],
                              in_=ob[:, QB:C])


def _build_program(with_bias=True):
    nc = bacc.Bacc("TRN2", target_bir_lowering=False, debug=False,
                   num_devices=NCORES)
    aps = {
        "xT": nc.dram_tensor("xT", [C, N], MMDT, kind="ExternalInput").ap(),
        # weights arrive pre-chunked: [128, KC*W] with chunk kc at cols
        # kc*W:(kc+1)*W   (host does the (6,128,W)->(128,6,W) transpose)
        "wqA": nc.dram_tensor("wqA", [128, KC * 128], MMDT,
                              kind="ExternalInput").ap(),
        "wkA": nc.dram_tensor("wkA", [128, KC * 128], MMDT,
                              kind="ExternalInput").ap(),
        "wqkB": nc.dram_tensor("wqkB", [128, KC * 128], MMDT,
                               kind="ExternalInput").ap(),
        "wv": nc.dram_tensor("wv", [128, KC * VW], MMDT,
                             kind="ExternalInput").ap(),
        "wp": nc.dram_tensor("wp", [MYC, C], MMDT, kind="ExternalInput").ap(),
        "bqA": nc.dram_tensor("bqA", [1, 128], MMDT,
                              kind="ExternalInput").ap(),
        "bkA": nc.dram_tensor("bkA", [1, 128], MMDT,
                              kind="ExternalInput").ap(),
        "bqkB": nc.dram_tensor("bqkB", [1, 128], MMDT,
                               kind="ExternalInput").ap(),
        "bvr": nc.dram_tensor("bvr", [1, VW], MMDT, kind="ExternalInput").ap(),
        "out": nc.dram_tensor("out", [N, C], MMDT,
                              kind="ExternalOutput").ap(),
    }
    with tile.TileContext(nc) as tc:
        import contextlib
        with contextlib.ExitStack() as ctx:
            pools = {
                "persist": ctx.enter_context(tc.tile_pool(name="persist", bufs=1)),
                "et": ctx.enter_context(tc.tile_pool(name="et", bufs=4)),
                "small": ctx.enter_context(tc.tile_pool(name="small", bufs=2)),
                "ostage": ctx.enter_context(tc.tile_pool(name="ostage", bufs=3)),
                "dram_bc": ctx.enter_context(
                    tc.tile_pool(name="dram_bc", bufs=2, space="DRAM")),
            }
            _emit(nc, tc, pools, aps, with_bias=with_bias)
    nc.compile()
    return nc


_PROGRAM_CACHE = {}


def _get_program(with_bias=True):
    key = f"nc{int(with_bias)}"
    if key not in _PROGRAM_CACHE:
        _PROGRAM_CACHE[key] = _build_program(with_bias=with_bias)
    return _PROGRAM_CACHE[key]


def _chunked(w):
    """[C, W] -> [128, KC*W]: chunk kc lands at columns kc*W:(kc+1)*W."""
    wc = np.ascontiguousarray(w)
    return wc.reshape(KC, 128, w.shape[1]).transpose(1, 0, 2).reshape(
        128, KC * w.shape[1])


def make_in_maps(x, Wq, bq, Wk, bk, Wv, bv, Wp, bp):
    scale = 1.0 / math.sqrt(DH)
    xTb = [np.ascontiguousarray(x[b].T) for b in range(B)]
    wire = mybir.dt.np(MMDT)
    in_maps = []
    for c in range(NCORES):
        b, hg = c // CPG, c % CPG
        cols = slice(hg * MYC, (hg + 1) * MYC)
        wq_c = Wq[:, cols] * np.float32(scale)
        wk_c = Wk[:, cols]
        wv_c = Wv[:, cols]
        # zero-gap wv: [v0 | 1-col | v1 | 1-col | v2 | 1-col]; bias row gets
        # the ones so psum comes out in v_sb layout directly
        wv_aug = np.zeros((C, VW), np.float32)
        bv_aug = np.zeros((1, VW), np.float32)
        for h in range(HPC):
            wv_aug[:, h * 65:h * 65 + 64] = wv_c[:, h * DH:(h + 1) * DH]
            bv_aug[0, h * 65:h * 65 + 64] = bv[cols][h * DH:(h + 1) * DH]
            bv_aug[0, h * 65 + 64] = 1.0
        in_maps.append({
            "xT": xTb[b].astype(wire),
            "wqA": _chunked(wq_c[:, 0:128]).astype(wire),
            "wkA": _chunked(wk_c[:, 0:128]).astype(wire),
            "wqkB": _chunked(np.concatenate([wq_c[:, 128:], wk_c[:, 128:]],
                                            axis=1)).astype(wire),
            "wv": _chunked(wv_aug).astype(wire),
            "wp": np.ascontiguousarray(Wp[cols, :]).astype(wire),
            "bqA": (bq[cols][0:128] * np.float32(scale)).reshape(1, 128)
                   .astype(wire),
            "bkA": bk[cols][0:128].reshape(1, 128).astype(wire),
            "bqkB": np.concatenate([bq[cols][128:] * np.float32(scale),
                                    bk[cols][128:]]).reshape(1, 128)
                    .astype(wire),
            "bvr": bv_aug.astype(wire),
        })
    return in_maps


def assemble(results, bp):
    out = np.empty((B, N, C), np.float32)
    for b in range(B):
        acc = results[b * CPG]["out"].astype(np.float64)
        for c in range(b * CPG + 1, (b + 1) * CPG):
            acc = acc + results[c]["out"]
        out[b] = (acc + bp.astype(np.float64)).astype(np.float32)
    return out


def kernel(x, Wq, bq, Wk, bk, Wv, bv, Wp, bp, **extra_kwargs):
    x = np.asarray(x, np.float32)
    Wq = np.asarray(Wq, np.float32)
    Wk = np.asarray(Wk, np.float32)
    Wv = np.asarray(Wv, np.float32)
    Wp = np.asarray(Wp, np.float32)
    bq = np.asarray(bq, np.float32)
    bk = np.asarray(bk, np.float32)
    bv = np.asarray(bv, np.float32)
    bp = np.asarray(bp, np.float32)

    with_bias = bool(np.any(bq) or np.any(bk))
    nc = _get_program(with_bias=with_bias)
    in_maps = make_in_maps(x, Wq, bq, Wk, bk, Wv, bv, Wp, bp)
    res = bass_utils.run_bass_kernel_spmd(nc, in_maps,
                                          core_ids=list(range(NCORES)))
    return assemble(res.results, bp)
